# revision 6
# baseline (speedup 1.0000x reference)
"""Trainium2 Bass kernel for classical self-attention (B=4, N=4096, D=1024, fp32).

  q = x @ Wq.T ; k = x @ Wk.T
  out = softmax(q @ k.T / sqrt(D)) @ x

Sharding: 8 cores = (batch b = c//2) x (query half h = c%2, 2048 queries each).
Each core computes its full softmax rows locally (holds all 4096 keys), so no
collectives are needed.

Device algorithm (per core), all matmuls in float32r (full PE rate at free>=256):
  phase 0:  kT = Wk.T-projection of x (transposed layout [D,N]) -> DRAM scratch
            qT = Wq.T-projection of the core's query slice     -> DRAM scratch
            (host supplies x.T and W.T so projections need no on-chip transposes)
  per 512-query block:
    phase A: scores computed directly TRANSPOSED: pT[n,q] = kT.T@qT chunks,
             exp via ScalarE (scale=1/32, no max-subtraction: scores ~ N(0,1)),
             row-sums s[q] via ones-vector matmuls (partition-dim reduction).
    phase B: out[q,:] accumulated over all 32 key chunks: lhsT = pT chunk,
             rhs = x natural chunks; final normalize by 1/s at PSUM eviction.
"""

import re

import numpy as np

import bass_rust
import concourse.bass as bass
import concourse.mybir as mybir
from concourse.tile import TileContext

B, N, D = 4, 4096, 1024
NQ = N // 2          # queries per core
QS = 512             # query super-block
P = 128              # partitions
DC = D // P          # 8 contraction chunks
F32 = mybir.dt.float32
F32R = mybir.dt.float32r
EXP = mybir.ActivationFunctionType.Exp
SCALE = 1.0 / 32.0   # 1/sqrt(D)


class SplitDrainTileContext(TileContext):
    """Workaround for this container's walrus codegen: an SP Drain with more
    than one sync wait fails ("Too many sync wait commands").  Emit the
    tail-drain waits as single-wait NOPs on the sync engine first, then a
    drain that needs no waits of its own."""

    def _drain_and_barrier(self, tick_clock, wait_clock):
        gc = tick_clock.global_clock
        ticks = [int(s) for s in re.findall(r"\d+", repr(gc))]
        for proc, t in enumerate(ticks):
            if t > 0:
                single = bass_rust.VectorClock()
                single.require_at_least(proc, t)
                nop = self.nc.sync.nop(nofuse=True, hint="split_drain_wait")
                wait_clock.add_sem_waits(nop.ins, bass_rust.ScopedClock({None: single}))
        drain_inst = self.nc.sync.drain()
        wait_clock.add_sem_waits(
            drain_inst.ins,
            bass_rust.ScopedClock({None: gc}),
            bass_rust.ScopedClock({None: gc.copy()}),
        )
        self.nc.all_engine_barrier()
        assert self.sems is not None
        popped = self.nc._tile_sem_poison_stack.pop()
        assert popped is self._sem_poison
        self.nc.clear_and_free_semaphores(list(self.sems.allocated().values()))
        self.nc.all_engine_barrier()


def _split_multiwaits(nc: bass.Bass, max_waits: int = 1) -> None:
    """This container's walrus codegen rejects instructions with more than one
    sync wait ("Too many sync wait commands").  Hoist extra waits onto
    injected NoOps placed immediately before the instruction in the same
    basic block (engines execute their stream in bb order, so the engine
    blocks on each NoOp's wait before reaching the real instruction)."""
    ctr = 0
    for bb in nc.main_func.blocks:
        new_list = []
        changed = False
        for inst in bb.instructions:
            si = inst.sync_info
            if si is not None and len(si.on_wait) > max_waits:
                waits = list(si.on_wait)
                keep = waits[-max_waits:]
                for w in waits[:-max_waits]:
                    nop = mybir.InstNoOp(name=f"splitw-{ctr}", ins=[], outs=[])
                    ctr += 1
                    nop.engine = inst.engine
                    nop.sync_info = mybir.SyncInfo(on_wait=[w], on_update=[])
                    new_list.append(nop)
                inst.sync_info = mybir.SyncInfo(
                    on_wait=keep, on_update=list(si.on_update)
                )
                changed = True
            new_list.append(inst)
        if changed:
            bb.instructions = new_list


def build_kernel() -> bass.Bass:
    nc = bass.Bass()
    x_nat = nc.dram_tensor("x", [N, D], F32R, kind="ExternalInput")
    xt = nc.dram_tensor("xt", [D, N], F32R, kind="ExternalInput")
    xtq = nc.dram_tensor("xtq", [D, NQ], F32R, kind="ExternalInput")
    wqt = nc.dram_tensor("wqt", [D, D], F32R, kind="ExternalInput")
    wkt = nc.dram_tensor("wkt", [D, D], F32R, kind="ExternalInput")
    out = nc.dram_tensor("out", [NQ, D], F32, kind="ExternalOutput")

    xt_r = xt.rearrange("(c p) n -> p c n", p=P)      # [128, 8, N]
    xtq_r = xtq.rearrange("(c p) n -> p c n", p=P)    # [128, 8, NQ]
    wqt_r = wqt.rearrange("(c p) e -> p c e", p=P)    # [128, 8, D]
    wkt_r = wkt.rearrange("(c p) e -> p c e", p=P)

    with SplitDrainTileContext(nc) as tc:
        with (
            tc.tile_pool(name="dram", bufs=1, space="DRAM") as dpool,
            tc.tile_pool(name="psum", bufs=8, space="PSUM") as pp,
            tc.tile_pool(name="persist", bufs=1) as persist,
        ):
            kt_d = dpool.tile([D, N], F32R, name="kt_d", tag="kt_d")
            qt_d = dpool.tile([D, NQ], F32R, name="qt_d", tag="qt_d")
            s_d = dpool.tile([NQ // QS, QS], F32, name="s_d", tag="s_d")
            kt_dr = kt_d.rearrange("(c p) n -> p c n", p=P)
            qt_dr = qt_d.rearrange("(c p) n -> p c n", p=P)

            ones_f32 = persist.tile([P, 1], F32, name="ones_f32", tag="ones32")
            nc.vector.memset(ones_f32, 1.0)
            ones_t = persist.tile([P, 1], F32R, name="ones_t", tag="ones")
            nc.scalar.copy(ones_t, ones_f32)

            # ---------------- phase 0: kT / qT projections -> DRAM ----------
            with (
                tc.tile_pool(name="wpool", bufs=1) as wpool,
                tc.tile_pool(name="p0x", bufs=2) as p0x,
                tc.tile_pool(name="p0c", bufs=4) as p0c,
            ):
                wk_sb = wpool.tile([P, DC, D], F32R, name="wk_sb", tag="wk")
                nc.sync.dma_start(out=wk_sb, in_=wkt_r)
                wq_sb = wpool.tile([P, DC, D], F32R, name="wq_sb", tag="wq")
                nc.sync.dma_start(out=wq_sb, in_=wqt_r)

                def project(w_sb, src_r, dst, nblk):
                    # dst[e, nb*512 : +512] = W.T-projection of src block
                    for nb in range(nblk):
                        xblk = p0x.tile([P, DC, QS], F32R, name="xblk", tag="xblk")
                        nc.sync.dma_start(
                            out=xblk, in_=src_r[:, :, nb * QS:(nb + 1) * QS]
                        )
                        for e in range(DC):
                            ps = pp.tile([P, QS], F32, name="ps0", tag="bank")
                            for d in range(DC):
                                nc.tensor.matmul(
                                    ps,
                                    w_sb[:, d, e * P:(e + 1) * P],
                                    xblk[:, d, :],
                                    start=(d == 0),
                                    stop=(d == DC - 1),
                                )
                            cp = p0c.tile([P, QS], F32R, name="cp", tag="cp")
                            nc.scalar.copy(cp, ps)
                            nc.sync.dma_start(
                                out=dst[e * P:(e + 1) * P, nb * QS:(nb + 1) * QS],
                                in_=cp,
                            )

                project(wk_sb, xt_r, kt_d, N // QS)
                project(wq_sb, xtq_r, qt_d, NQ // QS)

            # ---------------- main loop over 512-query super-blocks ---------
            with (
                tc.tile_pool(name="qtp", bufs=2) as qtp,
                tc.tile_pool(name="ktp", bufs=2) as ktp,
                tc.tile_pool(name="ptp", bufs=1) as ptp,
                tc.tile_pool(name="xbp", bufs=4) as xbp,
                tc.tile_pool(name="outp", bufs=4) as outp,
                tc.tile_pool(name="smallp", bufs=2) as smallp,
            ):
                NCH = N // P          # 32 key chunks
                NSTRIP = N // QS      # 8 key strips
                for qs in range(NQ // QS):
                    q0 = qs * QS
                    qt_strip = qtp.tile([P, DC, QS], F32R, name="qt_strip", tag="qt")
                    nc.sync.dma_start(out=qt_strip, in_=qt_dr[:, :, q0:q0 + QS])

                    # phase A: pT chunks + row-sums
                    pt_tiles = []
                    ps_s = pp.tile([1, QS], F32, name="ps_s", tag="bank")
                    for ns in range(NSTRIP):
                        kt_strip = ktp.tile([P, DC, QS], F32R, name="kt_strip", tag="kt")
                        nc.sync.dma_start(
                            out=kt_strip, in_=kt_dr[:, :, ns * QS:(ns + 1) * QS]
                        )
                        for j in range(QS // P):
                            nk = ns * (QS // P) + j
                            ps = pp.tile([P, QS], F32, name="ps_sc", tag="bank")
                            for e in range(DC):
                                nc.tensor.matmul(
                                    ps,
                                    kt_strip[:, e, j * P:(j + 1) * P],
                                    qt_strip[:, e, :],
                                    start=(e == 0),
                                    stop=(e == DC - 1),
                                )
                            pt = ptp.tile([P, QS], F32R, name="pt", tag=f"pt{nk}")
                            nc.scalar.activation(pt, ps, EXP, scale=SCALE)
                            pt_tiles.append(pt)
                            nc.tensor.matmul(
                                ps_s,
                                ones_t,
                                pt,
                                start=(nk == 0),
                                stop=(nk == NCH - 1),
                            )

                    # row-sum -> [128, 4] layout via tiny DRAM roundtrip
                    s_sb = smallp.tile([1, QS], F32, name="s_sb", tag="s_sb")
                    nc.scalar.copy(s_sb, ps_s)
                    nc.sync.dma_start(out=s_d[qs:qs + 1, :], in_=s_sb)
                    s_resh = smallp.tile([P, QS // P], F32, name="s_resh", tag="s_resh")
                    nc.sync.dma_start(
                        out=s_resh,
                        in_=s_d.rearrange("r (a p) -> r p a", p=P)[qs],
                    )
                    recip = smallp.tile([P, QS // P], F32, name="recip", tag="recip")
                    nc.vector.reciprocal(recip, s_resh)

                    # phase B: out accumulation over all key chunks
                    ps_o = [
                        pp.tile([P, QS], F32, name="ps_o", tag="bank")
                        for _ in range(8)
                    ]
                    for nk in range(NCH):
                        xc = xbp.tile([P, D], F32R, name="xc", tag="xc")
                        nc.sync.dma_start(out=xc, in_=x_nat[nk * P:(nk + 1) * P, :])
                        for qsub in range(QS // P):
                            lhsT = pt_tiles[nk][:, qsub * P:(qsub + 1) * P]
                            for eh in range(2):
                                nc.tensor.matmul(
                                    ps_o[qsub * 2 + eh],
                                    lhsT,
                                    xc[:, eh * QS:(eh + 1) * QS],
                                    start=(nk == 0),
                                    stop=(nk == NCH - 1),
                                )
                    for qsub in range(QS // P):
                        for eh in range(2):
                            o_sb = outp.tile([P, QS], F32, name="o_sb", tag="o_sb")
                            nc.vector.tensor_scalar_mul(
                                o_sb, ps_o[qsub * 2 + eh], recip[:, qsub:qsub + 1]
                            )
                            nc.sync.dma_start(
                                out=out[
                                    q0 + qsub * P:q0 + (qsub + 1) * P,
                                    eh * QS:(eh + 1) * QS,
                                ],
                                in_=o_sb,
                            )
    _split_multiwaits(nc)
    return nc


_NC_CACHE = None


def kernel(x: np.ndarray, Wq: np.ndarray, Wk: np.ndarray) -> np.ndarray:
    from concourse.bass_utils import run_bass_kernel_spmd

    global _NC_CACHE
    if _NC_CACHE is None:
        _NC_CACHE = build_kernel()
    nc = _NC_CACHE

    x = np.ascontiguousarray(x, dtype=np.float32)
    wqt = np.ascontiguousarray(Wq.T, dtype=np.float32)
    wkt = np.ascontiguousarray(Wk.T, dtype=np.float32)
    in_maps = []
    for c in range(8):
        b, h = divmod(c, 2)
        xb = np.ascontiguousarray(x[b])
        xtb = np.ascontiguousarray(x[b].T)
        in_maps.append(
            {
                "x": xb,
                "xt": xtb,
                "xtq": np.ascontiguousarray(xtb[:, h * NQ:(h + 1) * NQ]),
                "wqt": wqt,
                "wkt": wkt,
            }
        )

    res = run_bass_kernel_spmd(nc, in_maps, core_ids=list(range(8)))
    out = np.empty((B, N, D), dtype=np.float32)
    for c in range(8):
        b, h = divmod(c, 2)
        out[b, h * NQ:(h + 1) * NQ, :] = res.results[c]["out"]
    return out
